# revision 16
# baseline (speedup 1.0000x reference)
# CWIC MLP (gate-striped masked matmul + masked MLP) on 8 trn2 NeuronCores.
#
# Sharding: tensor-parallel over the `inter` dimension (32 stripes -> 4 per
# core). Each core computes z/up/h for its 1024 inter channels over ALL
# tokens, then a partial y[t, o]; partial y (plus a per-token active-params
# column) is summed across cores with chunked ReduceScatter collectives, so
# core c ends up owning a token shard of the final output.
#
# Per-core layouts: channel/inter-major ([i, t] / [s, t]) so per-channel
# thresholds/biases are per-partition scalars. Gate matmuls run in float32r
# (full-rate on TRN2, ~1.6e-4 rel err; gate precision controls mask_m flips),
# up/down matmuls in bf16. Gate masks are exact: compared in fp32 from
# host-computed xc = x - mu. Token chunks are software-pipelined: chunk ch+1's
# x/weight loads stream during chunk ch's down matmul.
import os
import numpy as np
import ml_dtypes

B, S, D_IN, D_INTER, D_OUT, STRIPE = 2, 1024, 2048, 8192, 2048, 256
NS = D_INTER // STRIPE          # 32 stripes
W = 8                           # cores
T = B * S                       # 2048 tokens
NSL = NS // W                   # 4 stripes per core
SL = D_INTER // W               # 1024 inter per core
STL = SL // 128                 # 8 local s-tiles
IT = D_IN // 128                # 16 i-tiles
NCH = 4                         # token chunks
TCH = T // NCH                  # 512 tokens per chunk
TT = TCH // 128                 # 4 t-tiles per chunk
OH = 2                          # o halves for the down matmul
OHW = D_OUT // OH               # 1024
YC = D_OUT + 1                  # y columns + active-params column
TSH = TCH // W                  # 64 tokens per core per chunk after RS

_cache = {}


def _build():
    import concourse.bacc as bacc
    import concourse.mybir as mybir
    import concourse.tile as tile
    from contextlib import ExitStack

    f32 = mybir.dt.float32
    f32r = mybir.dt.float32r
    bf16 = mybir.dt.bfloat16
    A = mybir.AluOpType
    AF = mybir.ActivationFunctionType

    nc = bacc.Bacc("TRN2", target_bir_lowering=False, debug=False, num_devices=W)

    xcT = nc.dram_tensor("xcT", [IT, 128, T], f32r, kind="ExternalInput")
    xcB = nc.dram_tensor("xcB", [IT, 128, T], bf16, kind="ExternalInput")
    wg = nc.dram_tensor("wg", [NSL, 128, IT, STRIPE], f32r, kind="ExternalInput")
    wu = nc.dram_tensor("wu", [NSL, 128, IT, STRIPE], bf16, kind="ExternalInput")
    dwt = nc.dram_tensor("dwt", [OH, STL, 128, OHW], bf16, kind="ExternalInput")
    thg = nc.dram_tensor("thg", [128, IT, NSL], f32, kind="ExternalInput")
    nthg = nc.dram_tensor("nthg", [128, IT, NSL], f32, kind="ExternalInput")
    pmu = nc.dram_tensor("pmu", [128, STL], f32, kind="ExternalInput")
    b2 = nc.dram_tensor("b2", [128, STL], f32, kind="ExternalInput")
    thm = nc.dram_tensor("thm", [128, STL], f32, kind="ExternalInput")
    yout = nc.dram_tensor("yout", [NCH, TSH, YC], f32, kind="ExternalOutput")

    bounce_in = nc.dram_tensor("bounce_in", [NCH, TCH, YC], f32)
    bounce_out = nc.dram_tensor("bounce_out", [NCH, TSH, YC], f32)

    with tile.TileContext(nc) as tc:
        with ExitStack() as ctx:
            cpool = ctx.enter_context(tc.tile_pool(name="consts", bufs=1))
            xcp = ctx.enter_context(tc.tile_pool(name="xc", bufs=1))
            ap_ = ctx.enter_context(tc.tile_pool(name="abs", bufs=1))
            wp = ctx.enter_context(tc.tile_pool(name="w", bufs=3))
            mp = ctx.enter_context(tc.tile_pool(name="m", bufs=5))
            xmp = ctx.enter_context(tc.tile_pool(name="xm", bufs=4))
            xbp = ctx.enter_context(tc.tile_pool(name="xcb", bufs=1))
            zsp = ctx.enter_context(tc.tile_pool(name="zs", bufs=2))
            usp = ctx.enter_context(tc.tile_pool(name="us", bufs=2))
            azp = ctx.enter_context(tc.tile_pool(name="az", bufs=4))
            mmp = ctx.enter_context(tc.tile_pool(name="mm", bufs=4))
            hp = ctx.enter_context(tc.tile_pool(name="h", bufs=2))
            dwp = ctx.enter_context(tc.tile_pool(name="dw", bufs=8))
            ysp = ctx.enter_context(tc.tile_pool(name="ysb", bufs=3))
            smp = ctx.enter_context(tc.tile_pool(name="small", bufs=4))

            thg_t = cpool.tile([128, IT, NSL], f32)
            nc.sync.dma_start(thg_t[:], thg.ap())
            nthg_t = cpool.tile([128, IT, NSL], f32)
            nc.sync.dma_start(nthg_t[:], nthg.ap())
            pmu_t = cpool.tile([128, STL], f32)
            nc.sync.dma_start(pmu_t[:], pmu.ap())
            b2_t = cpool.tile([128, STL], f32)
            nc.sync.dma_start(b2_t[:], b2.ap())
            thm_t = cpool.tile([128, STL], f32)
            nc.sync.dma_start(thm_t[:], thm.ap())
            ones_t = cpool.tile([128, 1], bf16)
            nc.vector.memset(ones_t[:], 1.0)
            sixt_t = cpool.tile([128, 1], bf16)
            nc.vector.memset(sixt_t[:], 16.0)

            def fetch_w(ch, n, wt):
                wgt = wp.tile([128, IT, STRIPE], f32r, tag="w",
                              name=f"wg_{ch}_{n}")
                nc.sync.dma_start(wgt[:], wg.ap()[n])
                wut = wp.tile([128, IT, STRIPE], bf16, tag="w",
                              name=f"wu_{ch}_{n}")
                nc.sync.dma_start(wut[:], wu.ap()[n])
                wt[n] = (wgt, wut)

            def emit_loads(ch):
                """Emit chunk ch's x loads and first two weight fetches, in
                sync-queue priority order (first-needed first)."""
                tsl = slice(ch * TCH, (ch + 1) * TCH)
                t = {"wt": {}, "ch": ch}
                fetch_w(ch, 0, t["wt"])
                xc_t = xcp.tile([128, IT, TCH], f32r, tag="xc",
                                name=f"xc_{ch}")
                xcb_t = xbp.tile([128, IT, TCH], bf16, tag="xcb",
                                 name=f"xcb_{ch}")
                for io in range(IT):
                    nc.sync.dma_start(xc_t[:, io, :], xcT.ap()[io][:, tsl])
                    nc.sync.dma_start(xcb_t[:, io, :], xcB.ap()[io][:, tsl])
                    if io == 7:
                        fetch_w(ch, 1, t["wt"])
                t["xc"] = xc_t
                t["xcb"] = xcb_t
                return t

            tiles = emit_loads(0)
            for ch in range(NCH):
                cur = tiles
                xc_t, xcb_t, wt = cur["xc"], cur["xcb"], cur["wt"]

                a_t = ap_.tile([128, IT, TCH], f32, tag="a", name=f"a_{ch}")
                for io in range(IT):
                    nc.scalar.activation(a_t[:, io, :],
                                         xc_t[:, io, :].bitcast(f32), AF.Abs)

                h_t = hp.tile([128, STL, TCH], bf16, tag="h", name=f"h_{ch}")

                with ExitStack() as pctx:
                    zps = pctx.enter_context(
                        tc.tile_pool(name="zps", bufs=3, space="PSUM"))
                    ups = pctx.enter_context(
                        tc.tile_pool(name="ups", bufs=3, space="PSUM"))
                    cps = pctx.enter_context(
                        tc.tile_pool(name="cps", bufs=1, space="PSUM"))

                    cnt_ps = cps.tile([1, TCH], f32, tag="cnt")

                    for n in range(NSL):
                        if n + 2 < NSL:
                            fetch_w(ch, n + 2, wt)
                        wgt, wut = wt.pop(n)

                        z0 = zps.tile([128, TCH], f32, tag="z")
                        z1 = zps.tile([128, TCH], f32, tag="z")
                        u0 = ups.tile([128, TCH], f32, tag="u")
                        u1 = ups.tile([128, TCH], f32, tag="u")

                        for io in range(IT):
                            m_t = mp.tile([128, TCH], bf16, tag="m")
                            if io % 4 == 3:
                                # ACT path: relu(a - th) then sign -> {0, 1}
                                r_t = azp.tile([128, TCH], f32, tag="tmp32")
                                nc.scalar.activation(
                                    r_t[:], a_t[:, io, :], AF.Relu,
                                    bias=nthg_t[:, io, n:n + 1])
                                nc.scalar.sign(m_t[:], r_t[:])
                            else:
                                nc.vector.tensor_scalar(
                                    m_t[:], a_t[:, io, :],
                                    thg_t[:, io, n:n + 1], None, A.is_gt)
                            xm_t = xmp.tile([128, TCH], f32r, tag="xm")
                            if io % 3 == 0:
                                nc.vector.tensor_tensor(
                                    xm_t[:], xc_t[:, io, :].bitcast(f32),
                                    m_t[:], A.mult)
                            else:
                                nc.gpsimd.tensor_tensor(
                                    xm_t[:], xc_t[:, io, :].bitcast(f32),
                                    m_t[:], A.mult)

                            st, sp = (io == 0), (io == IT - 1)
                            nc.tensor.matmul(z0[:], wgt[:, io, 0:128],
                                             xm_t[:], start=st, stop=sp)
                            nc.tensor.matmul(z1[:], wgt[:, io, 128:256],
                                             xm_t[:], start=st, stop=sp)
                            nc.tensor.matmul(cnt_ps[:], ones_t[:], m_t[:],
                                             start=(n == 0 and io == 0),
                                             stop=False)
                        for io in range(IT):
                            st, sp = (io == 0), (io == IT - 1)
                            nc.tensor.matmul(u0[:], wut[:, io, 0:128],
                                             xcb_t[:, io, :], start=st, stop=sp)
                            nc.tensor.matmul(u1[:], wut[:, io, 128:256],
                                             xcb_t[:, io, :], start=st, stop=sp)

                        for k, (zt, ut) in enumerate(((z0, u0), (z1, u1))):
                            gst = n * 2 + k
                            zs_t = zsp.tile([128, TCH], f32, tag="zs")
                            nc.scalar.activation(zs_t[:], zt[:], AF.Silu,
                                                 bias=pmu_t[:, gst:gst + 1])
                            us_t = usp.tile([128, TCH], f32, tag="us")
                            nc.scalar.add(us_t[:], ut[:], b2_t[:, gst:gst + 1])
                            az_t = azp.tile([128, TCH], f32, tag="tmp32")
                            nc.scalar.activation(az_t[:], zs_t[:], AF.Abs)
                            mm_t = mmp.tile([128, TCH], bf16, tag="mmk")
                            nc.vector.tensor_scalar(
                                mm_t[:], az_t[:], thm_t[:, gst:gst + 1],
                                None, A.is_gt)
                            t1_t = azp.tile([128, TCH], f32, tag="tmp32")
                            nc.gpsimd.tensor_tensor(t1_t[:], us_t[:], zs_t[:],
                                                    A.mult)
                            nc.vector.tensor_tensor(h_t[:, gst, :], t1_t[:],
                                                    mm_t[:], A.mult)
                            nc.tensor.matmul(
                                cnt_ps[:], sixt_t[:], mm_t[:], start=False,
                                stop=(n == NSL - 1 and k == 1))

                    act_sb = smp.tile([1, TCH], f32, tag="act")
                    nc.vector.tensor_scalar(act_sb[:], cnt_ps[:], 256.0, None,
                                            A.mult)
                    nc.scalar.dma_start(
                        bounce_in.ap()[ch, :, D_OUT:D_OUT + 1], act_sb[:])

                # Pre-issue this chunk's down-phase slabs, then next chunk's
                # loads -- deliberate sync-queue order: slabs first (needed in
                # ~1 matmul-phase), then ch+1's x/weights (stream during the
                # down matmuls and the RS).
                slabs = []
                for oh in range(OH):
                    for st in range(STL):
                        slab = dwp.tile([128, OHW], bf16, tag="dws",
                                        name=f"dws_{ch}_{oh}_{st}")
                        nc.sync.dma_start(slab[:], dwt.ap()[oh, st])
                        slabs.append(slab)
                if ch + 1 < NCH:
                    tiles = emit_loads(ch + 1)

                # ---- down matmul for this chunk: y[t, o] partial ----
                with ExitStack() as pctx:
                    yps = pctx.enter_context(
                        tc.tile_pool(name="yps", bufs=8, space="PSUM"))
                    for oh in range(OH):
                        yt = [[yps.tile([128, 512], f32,
                                        tag="y", name=f"y_{tt}_{oc}")
                               for oc in range(2)] for tt in range(TT)]
                        for st in range(STL):
                            slab = slabs[oh * STL + st]
                            for tt in range(TT):
                                lh = h_t[:, st, tt * 128:(tt + 1) * 128]
                                for oc in range(2):
                                    nc.tensor.matmul(
                                        yt[tt][oc][:], lh,
                                        slab[:, oc * 512:(oc + 1) * 512],
                                        start=(st == 0), stop=(st == STL - 1))
                        for tt in range(TT):
                            for oc in range(2):
                                ysb = ysp.tile([128, 512], f32, tag="ysb")
                                nc.scalar.copy(ysb[:], yt[tt][oc][:])
                                nc.scalar.dma_start(
                                    bounce_in.ap()[
                                        ch, tt * 128:(tt + 1) * 128,
                                        oh * OHW + oc * 512:
                                        oh * OHW + (oc + 1) * 512],
                                    ysb[:])

                nc.gpsimd.collective_compute(
                    "ReduceScatter",
                    A.add,
                    replica_groups=[list(range(W))],
                    ins=[bounce_in.ap()[ch]],
                    outs=[bounce_out.ap()[ch]],
                )

            # Drain RS results at the very end: these DMAs wait on collective
            # completion, and the Sync queue is FIFO -- placing them inside
            # the chunk loop stalls every later load behind the RS wait.
            for ch in range(NCH):
                nc.sync.dma_start(yout.ap()[ch], bounce_out.ap()[ch])

    nc.compile()
    return nc


def kernel(x, gate_weight, gate_thresholds, gate_mu, gate_std, up_weight,
           up_bias, mlp_thresholds, mlp_mad, down_weight):
    from concourse.bass_utils import run_bass_kernel_spmd

    x = np.asarray(x, np.float32)
    gate_weight = np.asarray(gate_weight, np.float32)
    gate_thresholds = np.asarray(gate_thresholds, np.float32)
    gate_mu = np.asarray(gate_mu, np.float32)
    gate_std = np.asarray(gate_std, np.float32)
    up_weight = np.asarray(up_weight, np.float32)
    up_bias = np.asarray(up_bias, np.float32)
    mlp_thresholds = np.asarray(mlp_thresholds, np.float32)
    mlp_mad = np.asarray(mlp_mad, np.float32)
    down_weight = np.asarray(down_weight, np.float32)

    if "nc" not in _cache:
        _cache["nc"] = _build()
    nc = _cache["nc"]

    xt = x.reshape(T, D_IN)
    xc = xt - gate_mu[None, :]                       # exact fp32, as reference
    xcT_blk = np.ascontiguousarray(xc.T.reshape(IT, 128, T))
    xcB_blk = xcT_blk.astype(ml_dtypes.bfloat16)

    thresh_g = gate_thresholds * gate_std[None, :]   # [NS, D_IN]
    post_mu = (gate_mu.astype(np.float64) @ gate_weight.astype(np.float64)
               ).astype(np.float32)
    bias2 = (up_bias.astype(np.float64)
             + gate_mu.astype(np.float64) @ up_weight.astype(np.float64)
             ).astype(np.float32)
    thm_full = mlp_thresholds * mlp_mad

    in_maps = []
    for c in range(W):
        sl = slice(c * SL, (c + 1) * SL)
        wgc = gate_weight[:, sl]
        wg_blk = np.ascontiguousarray(
            wgc.reshape(IT, 128, NSL, STRIPE).transpose(2, 1, 0, 3))
        wuc = up_weight[:, sl]
        wu_blk = np.ascontiguousarray(
            wuc.reshape(IT, 128, NSL, STRIPE).transpose(2, 1, 0, 3)
            .astype(ml_dtypes.bfloat16))
        dwc = down_weight[:, sl]                     # [D_OUT, SL]
        dwt_blk = np.ascontiguousarray(
            dwc.T.reshape(STL, 128, OH, OHW).transpose(2, 0, 1, 3)
            .astype(ml_dtypes.bfloat16))
        thg_blk = np.ascontiguousarray(
            thresh_g[c * NSL:(c + 1) * NSL]
            .reshape(NSL, IT, 128).transpose(2, 1, 0))
        nthg_blk = np.ascontiguousarray(-thg_blk)
        pmu_blk = np.ascontiguousarray(post_mu[sl].reshape(STL, 128).T)
        b2_blk = np.ascontiguousarray(bias2[sl].reshape(STL, 128).T)
        thm_blk = np.ascontiguousarray(thm_full[sl].reshape(STL, 128).T)
        in_maps.append(dict(
            xcT=xcT_blk, xcB=xcB_blk, wg=wg_blk, wu=wu_blk, dwt=dwt_blk,
            thg=thg_blk, nthg=nthg_blk, pmu=pmu_blk, b2=b2_blk, thm=thm_blk))

    trace = bool(os.environ.get("CWIC_TRACE"))
    res = run_bass_kernel_spmd(nc, in_maps, core_ids=list(range(W)),
                               trace=trace)
    if trace and res.exec_time_ns is not None:
        print(f"HW exec time: {res.exec_time_ns} ns")
        _cache["exec_time_ns"] = res.exec_time_ns
        _cache["trace"] = res.instructions_and_trace

    y = np.empty((T, D_OUT), np.float32)
    active = np.empty((T,), np.float32)
    for ch in range(NCH):
        for c in range(W):
            rows = slice(ch * TCH + c * TSH, ch * TCH + (c + 1) * TSH)
            shard = res.results[c]["yout"][ch]
            y[rows] = shard[:, :D_OUT]
            active[rows] = shard[:, D_OUT]

    dense_val = float(D_IN * D_INTER) + float((D_IN + D_OUT) * D_INTER)
    dense = np.full((B, S), dense_val, np.float32)
    return (y.reshape(B, S, D_OUT), dense, active.reshape(B, S))


# revision 17
# speedup vs baseline: 1.0158x; 1.0158x over previous
# CWIC MLP (gate-striped masked matmul + masked MLP) on 8 trn2 NeuronCores.
#
# Sharding: tensor-parallel over the `inter` dimension (32 stripes -> 4 per
# core). Each core computes z/up/h for its 1024 inter channels over ALL
# tokens, then a partial y[t, o]; partial y (plus a per-token active-params
# column) is summed across cores with chunked ReduceScatter collectives, so
# core c ends up owning a token shard of the final output.
#
# Per-core layouts: channel/inter-major ([i, t] / [s, t]) so per-channel
# thresholds/biases are per-partition scalars. Gate matmuls run in float32r
# (full-rate on TRN2, ~1.6e-4 rel err; gate precision controls mask_m flips),
# up/down matmuls in bf16. Gate masks are exact: compared in fp32 from
# host-computed xc = x - mu. Token chunks are software-pipelined: chunk ch+1's
# x/weight loads stream during chunk ch's down matmul.
import os
import numpy as np
import ml_dtypes

B, S, D_IN, D_INTER, D_OUT, STRIPE = 2, 1024, 2048, 8192, 2048, 256
NS = D_INTER // STRIPE          # 32 stripes
W = 8                           # cores
T = B * S                       # 2048 tokens
NSL = NS // W                   # 4 stripes per core
SL = D_INTER // W               # 1024 inter per core
STL = SL // 128                 # 8 local s-tiles
IT = D_IN // 128                # 16 i-tiles
NCH = 4                         # token chunks
TCH = T // NCH                  # 512 tokens per chunk
TT = TCH // 128                 # 4 t-tiles per chunk
OH = 2                          # o halves for the down matmul
OHW = D_OUT // OH               # 1024
YC = D_OUT + 1                  # y columns + active-params column
TSH = TCH // W                  # 64 tokens per core per chunk after RS

_cache = {}


def _build():
    import concourse.bacc as bacc
    import concourse.mybir as mybir
    import concourse.tile as tile
    from contextlib import ExitStack

    f32 = mybir.dt.float32
    f32r = mybir.dt.float32r
    bf16 = mybir.dt.bfloat16
    A = mybir.AluOpType
    AF = mybir.ActivationFunctionType

    nc = bacc.Bacc("TRN2", target_bir_lowering=False, debug=False, num_devices=W)

    xcT = nc.dram_tensor("xcT", [IT, 128, T], f32r, kind="ExternalInput")
    xcB = nc.dram_tensor("xcB", [IT, 128, T], bf16, kind="ExternalInput")
    wg = nc.dram_tensor("wg", [NSL, 128, IT, STRIPE], f32r, kind="ExternalInput")
    wu = nc.dram_tensor("wu", [NSL, 128, IT, STRIPE], bf16, kind="ExternalInput")
    dwt = nc.dram_tensor("dwt", [OH, STL, 128, OHW], bf16, kind="ExternalInput")
    thg = nc.dram_tensor("thg", [128, IT, NSL], f32, kind="ExternalInput")
    nthg = nc.dram_tensor("nthg", [128, IT, NSL], f32, kind="ExternalInput")
    pmu = nc.dram_tensor("pmu", [128, STL], f32, kind="ExternalInput")
    b2 = nc.dram_tensor("b2", [128, STL], f32, kind="ExternalInput")
    thm = nc.dram_tensor("thm", [128, STL], f32, kind="ExternalInput")
    yout = nc.dram_tensor("yout", [NCH, TSH, YC], f32, kind="ExternalOutput")

    bounce_in = nc.dram_tensor("bounce_in", [NCH, TCH, YC], f32)
    bounce_out = nc.dram_tensor("bounce_out", [NCH, TSH, YC], f32)

    with tile.TileContext(nc) as tc:
        with ExitStack() as ctx:
            cpool = ctx.enter_context(tc.tile_pool(name="consts", bufs=1))
            xcp = ctx.enter_context(tc.tile_pool(name="xc", bufs=1))
            ap_ = ctx.enter_context(tc.tile_pool(name="abs", bufs=1))
            wp = ctx.enter_context(tc.tile_pool(name="w", bufs=3))
            mp = ctx.enter_context(tc.tile_pool(name="m", bufs=5))
            xmp = ctx.enter_context(tc.tile_pool(name="xm", bufs=4))
            xbp = ctx.enter_context(tc.tile_pool(name="xcb", bufs=1))
            zsp = ctx.enter_context(tc.tile_pool(name="zs", bufs=2))
            usp = ctx.enter_context(tc.tile_pool(name="us", bufs=2))
            azp = ctx.enter_context(tc.tile_pool(name="az", bufs=4))
            mmp = ctx.enter_context(tc.tile_pool(name="mm", bufs=4))
            hp = ctx.enter_context(tc.tile_pool(name="h", bufs=2))
            dwp = ctx.enter_context(tc.tile_pool(name="dw", bufs=8))
            ysp = ctx.enter_context(tc.tile_pool(name="ysb", bufs=3))
            smp = ctx.enter_context(tc.tile_pool(name="small", bufs=4))

            thg_t = cpool.tile([128, IT, NSL], f32)
            nc.sync.dma_start(thg_t[:], thg.ap())
            nthg_t = cpool.tile([128, IT, NSL], f32)
            nc.sync.dma_start(nthg_t[:], nthg.ap())
            pmu_t = cpool.tile([128, STL], f32)
            nc.sync.dma_start(pmu_t[:], pmu.ap())
            b2_t = cpool.tile([128, STL], f32)
            nc.sync.dma_start(b2_t[:], b2.ap())
            thm_t = cpool.tile([128, STL], f32)
            nc.sync.dma_start(thm_t[:], thm.ap())
            ones_t = cpool.tile([128, 1], bf16)
            nc.vector.memset(ones_t[:], 1.0)
            sixt_t = cpool.tile([128, 1], bf16)
            nc.vector.memset(sixt_t[:], 16.0)

            def fetch_w(ch, n, wt):
                wgt = wp.tile([128, IT, STRIPE], f32r, tag="w",
                              name=f"wg_{ch}_{n}")
                nc.sync.dma_start(wgt[:], wg.ap()[n])
                wut = wp.tile([128, IT, STRIPE], bf16, tag="w",
                              name=f"wu_{ch}_{n}")
                nc.sync.dma_start(wut[:], wu.ap()[n])
                wt[n] = (wgt, wut)

            def emit_loads(ch):
                """Emit chunk ch's x loads and first two weight fetches, in
                sync-queue priority order (first-needed first)."""
                tsl = slice(ch * TCH, (ch + 1) * TCH)
                t = {"wt": {}, "ch": ch}
                fetch_w(ch, 0, t["wt"])
                xc_t = xcp.tile([128, IT, TCH], f32r, tag="xc",
                                name=f"xc_{ch}")
                xcb_t = xbp.tile([128, IT, TCH], bf16, tag="xcb",
                                 name=f"xcb_{ch}")
                for io in range(IT):
                    nc.sync.dma_start(xc_t[:, io, :], xcT.ap()[io][:, tsl])
                    nc.gpsimd.dma_start(xcb_t[:, io, :], xcB.ap()[io][:, tsl])
                    if io == 7:
                        fetch_w(ch, 1, t["wt"])
                t["xc"] = xc_t
                t["xcb"] = xcb_t
                return t

            tiles = emit_loads(0)
            for ch in range(NCH):
                cur = tiles
                xc_t, xcb_t, wt = cur["xc"], cur["xcb"], cur["wt"]

                a_t = ap_.tile([128, IT, TCH], f32, tag="a", name=f"a_{ch}")
                for io in range(IT):
                    nc.scalar.activation(a_t[:, io, :],
                                         xc_t[:, io, :].bitcast(f32), AF.Abs)

                h_t = hp.tile([128, STL, TCH], bf16, tag="h", name=f"h_{ch}")

                with ExitStack() as pctx:
                    zps = pctx.enter_context(
                        tc.tile_pool(name="zps", bufs=3, space="PSUM"))
                    ups = pctx.enter_context(
                        tc.tile_pool(name="ups", bufs=3, space="PSUM"))
                    cps = pctx.enter_context(
                        tc.tile_pool(name="cps", bufs=1, space="PSUM"))

                    cnt_ps = cps.tile([1, TCH], f32, tag="cnt")

                    for n in range(NSL):
                        if n + 2 < NSL:
                            fetch_w(ch, n + 2, wt)
                        wgt, wut = wt.pop(n)

                        z0 = zps.tile([128, TCH], f32, tag="z")
                        z1 = zps.tile([128, TCH], f32, tag="z")
                        u0 = ups.tile([128, TCH], f32, tag="u")
                        u1 = ups.tile([128, TCH], f32, tag="u")

                        for io in range(IT):
                            m_t = mp.tile([128, TCH], bf16, tag="m")
                            if io % 4 == 3:
                                # ACT path: relu(a - th) then sign -> {0, 1}
                                r_t = azp.tile([128, TCH], f32, tag="tmp32")
                                nc.scalar.activation(
                                    r_t[:], a_t[:, io, :], AF.Relu,
                                    bias=nthg_t[:, io, n:n + 1])
                                nc.scalar.sign(m_t[:], r_t[:])
                            else:
                                nc.vector.tensor_scalar(
                                    m_t[:], a_t[:, io, :],
                                    thg_t[:, io, n:n + 1], None, A.is_gt)
                            xm_t = xmp.tile([128, TCH], f32r, tag="xm")
                            if io % 3 == 0:
                                nc.vector.tensor_tensor(
                                    xm_t[:], xc_t[:, io, :].bitcast(f32),
                                    m_t[:], A.mult)
                            else:
                                nc.gpsimd.tensor_tensor(
                                    xm_t[:], xc_t[:, io, :].bitcast(f32),
                                    m_t[:], A.mult)

                            st, sp = (io == 0), (io == IT - 1)
                            nc.tensor.matmul(z0[:], wgt[:, io, 0:128],
                                             xm_t[:], start=st, stop=sp)
                            nc.tensor.matmul(z1[:], wgt[:, io, 128:256],
                                             xm_t[:], start=st, stop=sp)
                            nc.tensor.matmul(cnt_ps[:], ones_t[:], m_t[:],
                                             start=(n == 0 and io == 0),
                                             stop=False)
                        for io in range(IT):
                            st, sp = (io == 0), (io == IT - 1)
                            nc.tensor.matmul(u0[:], wut[:, io, 0:128],
                                             xcb_t[:, io, :], start=st, stop=sp)
                            nc.tensor.matmul(u1[:], wut[:, io, 128:256],
                                             xcb_t[:, io, :], start=st, stop=sp)

                        for k, (zt, ut) in enumerate(((z0, u0), (z1, u1))):
                            gst = n * 2 + k
                            zs_t = zsp.tile([128, TCH], f32, tag="zs")
                            nc.scalar.activation(zs_t[:], zt[:], AF.Silu,
                                                 bias=pmu_t[:, gst:gst + 1])
                            us_t = usp.tile([128, TCH], f32, tag="us")
                            nc.scalar.add(us_t[:], ut[:], b2_t[:, gst:gst + 1])
                            az_t = azp.tile([128, TCH], f32, tag="tmp32")
                            nc.scalar.activation(az_t[:], zs_t[:], AF.Abs)
                            mm_t = mmp.tile([128, TCH], bf16, tag="mmk")
                            nc.vector.tensor_scalar(
                                mm_t[:], az_t[:], thm_t[:, gst:gst + 1],
                                None, A.is_gt)
                            t1_t = azp.tile([128, TCH], f32, tag="tmp32")
                            nc.gpsimd.tensor_tensor(t1_t[:], us_t[:], zs_t[:],
                                                    A.mult)
                            nc.vector.tensor_tensor(h_t[:, gst, :], t1_t[:],
                                                    mm_t[:], A.mult)
                            nc.tensor.matmul(
                                cnt_ps[:], sixt_t[:], mm_t[:], start=False,
                                stop=(n == NSL - 1 and k == 1))

                    act_sb = smp.tile([1, TCH], f32, tag="act")
                    nc.vector.tensor_scalar(act_sb[:], cnt_ps[:], 256.0, None,
                                            A.mult)
                    nc.scalar.dma_start(
                        bounce_in.ap()[ch, :, D_OUT:D_OUT + 1], act_sb[:])

                # ---- down matmul for this chunk: y[t, o] partial ----
                with ExitStack() as pctx:
                    yps = pctx.enter_context(
                        tc.tile_pool(name="yps", bufs=8, space="PSUM"))
                    for oh in range(OH):
                        yt = [[yps.tile([128, 512], f32,
                                        tag="y", name=f"y_{tt}_{oc}")
                               for oc in range(2)] for tt in range(TT)]
                        for st in range(STL):
                            slab = dwp.tile([128, OHW], bf16, tag="dws",
                                            name=f"dws_{ch}_{oh}_{st}")
                            nc.sync.dma_start(slab[:], dwt.ap()[oh, st])
                            for tt in range(TT):
                                lh = h_t[:, st, tt * 128:(tt + 1) * 128]
                                for oc in range(2):
                                    nc.tensor.matmul(
                                        yt[tt][oc][:], lh,
                                        slab[:, oc * 512:(oc + 1) * 512],
                                        start=(st == 0), stop=(st == STL - 1))
                        for tt in range(TT):
                            for oc in range(2):
                                ysb = ysp.tile([128, 512], f32, tag="ysb")
                                nc.scalar.copy(ysb[:], yt[tt][oc][:])
                                nc.scalar.dma_start(
                                    bounce_in.ap()[
                                        ch, tt * 128:(tt + 1) * 128,
                                        oh * OHW + oc * 512:
                                        oh * OHW + (oc + 1) * 512],
                                    ysb[:])

                nc.gpsimd.collective_compute(
                    "ReduceScatter",
                    A.add,
                    replica_groups=[list(range(W))],
                    ins=[bounce_in.ap()[ch]],
                    outs=[bounce_out.ap()[ch]],
                )
                if ch + 1 < NCH:
                    tiles = emit_loads(ch + 1)

            # Drain RS results at the very end: these DMAs wait on collective
            # completion, and the Sync queue is FIFO -- placing them inside
            # the chunk loop stalls every later load behind the RS wait.
            for ch in range(NCH):
                nc.sync.dma_start(yout.ap()[ch], bounce_out.ap()[ch])

    nc.compile()
    return nc


def kernel(x, gate_weight, gate_thresholds, gate_mu, gate_std, up_weight,
           up_bias, mlp_thresholds, mlp_mad, down_weight):
    from concourse.bass_utils import run_bass_kernel_spmd

    x = np.asarray(x, np.float32)
    gate_weight = np.asarray(gate_weight, np.float32)
    gate_thresholds = np.asarray(gate_thresholds, np.float32)
    gate_mu = np.asarray(gate_mu, np.float32)
    gate_std = np.asarray(gate_std, np.float32)
    up_weight = np.asarray(up_weight, np.float32)
    up_bias = np.asarray(up_bias, np.float32)
    mlp_thresholds = np.asarray(mlp_thresholds, np.float32)
    mlp_mad = np.asarray(mlp_mad, np.float32)
    down_weight = np.asarray(down_weight, np.float32)

    if "nc" not in _cache:
        _cache["nc"] = _build()
    nc = _cache["nc"]

    xt = x.reshape(T, D_IN)
    xc = xt - gate_mu[None, :]                       # exact fp32, as reference
    xcT_blk = np.ascontiguousarray(xc.T.reshape(IT, 128, T))
    xcB_blk = xcT_blk.astype(ml_dtypes.bfloat16)

    thresh_g = gate_thresholds * gate_std[None, :]   # [NS, D_IN]
    post_mu = (gate_mu.astype(np.float64) @ gate_weight.astype(np.float64)
               ).astype(np.float32)
    bias2 = (up_bias.astype(np.float64)
             + gate_mu.astype(np.float64) @ up_weight.astype(np.float64)
             ).astype(np.float32)
    thm_full = mlp_thresholds * mlp_mad

    in_maps = []
    for c in range(W):
        sl = slice(c * SL, (c + 1) * SL)
        wgc = gate_weight[:, sl]
        wg_blk = np.ascontiguousarray(
            wgc.reshape(IT, 128, NSL, STRIPE).transpose(2, 1, 0, 3))
        wuc = up_weight[:, sl]
        wu_blk = np.ascontiguousarray(
            wuc.reshape(IT, 128, NSL, STRIPE).transpose(2, 1, 0, 3)
            .astype(ml_dtypes.bfloat16))
        dwc = down_weight[:, sl]                     # [D_OUT, SL]
        dwt_blk = np.ascontiguousarray(
            dwc.T.reshape(STL, 128, OH, OHW).transpose(2, 0, 1, 3)
            .astype(ml_dtypes.bfloat16))
        thg_blk = np.ascontiguousarray(
            thresh_g[c * NSL:(c + 1) * NSL]
            .reshape(NSL, IT, 128).transpose(2, 1, 0))
        nthg_blk = np.ascontiguousarray(-thg_blk)
        pmu_blk = np.ascontiguousarray(post_mu[sl].reshape(STL, 128).T)
        b2_blk = np.ascontiguousarray(bias2[sl].reshape(STL, 128).T)
        thm_blk = np.ascontiguousarray(thm_full[sl].reshape(STL, 128).T)
        in_maps.append(dict(
            xcT=xcT_blk, xcB=xcB_blk, wg=wg_blk, wu=wu_blk, dwt=dwt_blk,
            thg=thg_blk, nthg=nthg_blk, pmu=pmu_blk, b2=b2_blk, thm=thm_blk))

    trace = bool(os.environ.get("CWIC_TRACE"))
    res = run_bass_kernel_spmd(nc, in_maps, core_ids=list(range(W)),
                               trace=trace)
    if trace and res.exec_time_ns is not None:
        print(f"HW exec time: {res.exec_time_ns} ns")
        _cache["exec_time_ns"] = res.exec_time_ns
        _cache["trace"] = res.instructions_and_trace

    y = np.empty((T, D_OUT), np.float32)
    active = np.empty((T,), np.float32)
    for ch in range(NCH):
        for c in range(W):
            rows = slice(ch * TCH + c * TSH, ch * TCH + (c + 1) * TSH)
            shard = res.results[c]["yout"][ch]
            y[rows] = shard[:, :D_OUT]
            active[rows] = shard[:, D_OUT]

    dense_val = float(D_IN * D_INTER) + float((D_IN + D_OUT) * D_INTER)
    dense = np.full((B, S), dense_val, np.float32)
    return (y.reshape(B, S, D_OUT), dense, active.reshape(B, S))


# revision 18
# speedup vs baseline: 1.0353x; 1.0192x over previous
# CWIC MLP (gate-striped masked matmul + masked MLP) on 8 trn2 NeuronCores.
#
# Sharding: tensor-parallel over the `inter` dimension (32 stripes -> 4 per
# core). Each core computes z/up/h for its 1024 inter channels over ALL
# tokens, then a partial y[t, o]; partial y (plus a per-token active-params
# column) is summed across cores with chunked ReduceScatter collectives, so
# core c ends up owning a token shard of the final output.
#
# Per-core layouts: channel/inter-major ([i, t] / [s, t]) so per-channel
# thresholds/biases are per-partition scalars. Gate matmuls run in float32r
# (full-rate on TRN2, ~1.6e-4 rel err; gate precision controls mask_m flips),
# up/down matmuls in bf16. Gate masks are exact: compared in fp32 from
# host-computed xc = x - mu. Token chunks are software-pipelined: chunk ch+1's
# x/weight loads stream during chunk ch's down matmul.
import os
import numpy as np
import ml_dtypes

B, S, D_IN, D_INTER, D_OUT, STRIPE = 2, 1024, 2048, 8192, 2048, 256
NS = D_INTER // STRIPE          # 32 stripes
W = 8                           # cores
T = B * S                       # 2048 tokens
NSL = NS // W                   # 4 stripes per core
SL = D_INTER // W               # 1024 inter per core
STL = SL // 128                 # 8 local s-tiles
IT = D_IN // 128                # 16 i-tiles
NCH = 4                         # token chunks
TCH = T // NCH                  # 512 tokens per chunk
TT = TCH // 128                 # 4 t-tiles per chunk
OH = 2                          # o halves for the down matmul
OHW = D_OUT // OH               # 1024
YC = D_OUT + 1                  # y columns + active-params column
TSH = TCH // W                  # 64 tokens per core per chunk after RS

_cache = {}


def _build():
    import concourse.bacc as bacc
    import concourse.mybir as mybir
    import concourse.tile as tile
    from contextlib import ExitStack

    f32 = mybir.dt.float32
    f32r = mybir.dt.float32r
    bf16 = mybir.dt.bfloat16
    A = mybir.AluOpType
    AF = mybir.ActivationFunctionType

    nc = bacc.Bacc("TRN2", target_bir_lowering=False, debug=False, num_devices=W)

    xcT = nc.dram_tensor("xcT", [IT, 128, T], f32r, kind="ExternalInput")
    xcB = nc.dram_tensor("xcB", [IT, 128, T], bf16, kind="ExternalInput")
    wg = nc.dram_tensor("wg", [NSL, 128, IT, STRIPE], f32r, kind="ExternalInput")
    wu = nc.dram_tensor("wu", [NSL, 128, IT, STRIPE], bf16, kind="ExternalInput")
    dwt = nc.dram_tensor("dwt", [OH, STL, 128, OHW], bf16, kind="ExternalInput")
    thg = nc.dram_tensor("thg", [128, IT, NSL], f32, kind="ExternalInput")
    nthg = nc.dram_tensor("nthg", [128, IT, NSL], f32, kind="ExternalInput")
    pmu = nc.dram_tensor("pmu", [128, STL], f32, kind="ExternalInput")
    b2 = nc.dram_tensor("b2", [128, STL], f32, kind="ExternalInput")
    thm = nc.dram_tensor("thm", [128, STL], f32, kind="ExternalInput")
    yout = nc.dram_tensor("yout", [NCH, TSH, YC], f32, kind="ExternalOutput")

    bounce_in = nc.dram_tensor("bounce_in", [NCH, TCH, YC], f32)
    bounce_out = nc.dram_tensor("bounce_out", [NCH, TSH, YC], f32)

    with tile.TileContext(nc) as tc:
        with ExitStack() as ctx:
            cpool = ctx.enter_context(tc.tile_pool(name="consts", bufs=1))
            xcp = ctx.enter_context(tc.tile_pool(name="xc", bufs=1))
            ap_ = ctx.enter_context(tc.tile_pool(name="abs", bufs=1))
            wp = ctx.enter_context(tc.tile_pool(name="w", bufs=3))
            mp = ctx.enter_context(tc.tile_pool(name="m", bufs=5))
            xmp = ctx.enter_context(tc.tile_pool(name="xm", bufs=4))
            xbp = ctx.enter_context(tc.tile_pool(name="xcb", bufs=1))
            zsp = ctx.enter_context(tc.tile_pool(name="zs", bufs=2))
            usp = ctx.enter_context(tc.tile_pool(name="us", bufs=2))
            azp = ctx.enter_context(tc.tile_pool(name="az", bufs=4))
            mmp = ctx.enter_context(tc.tile_pool(name="mm", bufs=4))
            hp = ctx.enter_context(tc.tile_pool(name="h", bufs=2))
            dwp = ctx.enter_context(tc.tile_pool(name="dw", bufs=8))
            ysp = ctx.enter_context(tc.tile_pool(name="ysb", bufs=3))
            smp = ctx.enter_context(tc.tile_pool(name="small", bufs=4))

            thg_t = cpool.tile([128, IT, NSL], f32)
            nc.sync.dma_start(thg_t[:], thg.ap())
            nthg_t = cpool.tile([128, IT, NSL], f32)
            nc.sync.dma_start(nthg_t[:], nthg.ap())
            pmu_t = cpool.tile([128, STL], f32)
            nc.sync.dma_start(pmu_t[:], pmu.ap())
            b2_t = cpool.tile([128, STL], f32)
            nc.sync.dma_start(b2_t[:], b2.ap())
            thm_t = cpool.tile([128, STL], f32)
            nc.sync.dma_start(thm_t[:], thm.ap())
            ones_t = cpool.tile([128, 1], bf16)
            nc.vector.memset(ones_t[:], 1.0)
            sixt_t = cpool.tile([128, 1], bf16)
            nc.vector.memset(sixt_t[:], 16.0)

            def fetch_w(ch, n, wt):
                wgt = wp.tile([128, IT, STRIPE], f32r, tag="w",
                              name=f"wg_{ch}_{n}")
                nc.sync.dma_start(wgt[:], wg.ap()[n])
                wut = wp.tile([128, IT, STRIPE], bf16, tag="w",
                              name=f"wu_{ch}_{n}")
                nc.sync.dma_start(wut[:], wu.ap()[n])
                wt[n] = (wgt, wut)

            def emit_loads(ch):
                """Emit chunk ch's x loads and first two weight fetches, in
                sync-queue priority order (first-needed first)."""
                tsl = slice(ch * TCH, (ch + 1) * TCH)
                t = {"wt": {}, "ch": ch}
                fetch_w(ch, 0, t["wt"])
                xc_t = xcp.tile([128, IT, TCH], f32r, tag="xc",
                                name=f"xc_{ch}")
                xcb_t = xbp.tile([128, IT, TCH], bf16, tag="xcb",
                                 name=f"xcb_{ch}")
                for io in range(IT):
                    nc.sync.dma_start(xc_t[:, io, :], xcT.ap()[io][:, tsl])
                    nc.gpsimd.dma_start(xcb_t[:, io, :], xcB.ap()[io][:, tsl])
                    if io == 7:
                        fetch_w(ch, 1, t["wt"])
                t["xc"] = xc_t
                t["xcb"] = xcb_t
                return t

            tiles = emit_loads(0)
            pending_rs = None
            for ch in range(NCH):
                cur = tiles
                xc_t, xcb_t, wt = cur["xc"], cur["xcb"], cur["wt"]

                a_t = ap_.tile([128, IT, TCH], f32, tag="a", name=f"a_{ch}")
                for io in range(IT):
                    nc.scalar.activation(a_t[:, io, :],
                                         xc_t[:, io, :].bitcast(f32), AF.Abs)

                h_t = hp.tile([128, STL, TCH], bf16, tag="h", name=f"h_{ch}")

                with ExitStack() as pctx:
                    zps = pctx.enter_context(
                        tc.tile_pool(name="zps", bufs=3, space="PSUM"))
                    ups = pctx.enter_context(
                        tc.tile_pool(name="ups", bufs=3, space="PSUM"))
                    cps = pctx.enter_context(
                        tc.tile_pool(name="cps", bufs=1, space="PSUM"))

                    cnt_ps = cps.tile([1, TCH], f32, tag="cnt")

                    for n in range(NSL):
                        if n == 2 and pending_rs is not None:
                            # Emit previous chunk's ReduceScatter here: its
                            # input deps completed during our stripes 0-1, so
                            # the trigger retires without blocking the GpSimd
                            # queue (emitting it right after the down phase
                            # stalls all of this chunk's Pool work behind it).
                            nc.gpsimd.collective_compute(
                                "ReduceScatter", A.add,
                                replica_groups=[list(range(W))],
                                ins=[bounce_in.ap()[pending_rs]],
                                outs=[bounce_out.ap()[pending_rs]])
                            pending_rs = None
                        if n + 2 < NSL:
                            fetch_w(ch, n + 2, wt)
                        wgt, wut = wt.pop(n)

                        z0 = zps.tile([128, TCH], f32, tag="z")
                        z1 = zps.tile([128, TCH], f32, tag="z")
                        u0 = ups.tile([128, TCH], f32, tag="u")
                        u1 = ups.tile([128, TCH], f32, tag="u")

                        for io in range(IT):
                            m_t = mp.tile([128, TCH], bf16, tag="m")
                            if io % 4 == 3:
                                # ACT path: relu(a - th) then sign -> {0, 1}
                                r_t = azp.tile([128, TCH], f32, tag="tmp32")
                                nc.scalar.activation(
                                    r_t[:], a_t[:, io, :], AF.Relu,
                                    bias=nthg_t[:, io, n:n + 1])
                                nc.scalar.sign(m_t[:], r_t[:])
                            else:
                                nc.vector.tensor_scalar(
                                    m_t[:], a_t[:, io, :],
                                    thg_t[:, io, n:n + 1], None, A.is_gt)
                            xm_t = xmp.tile([128, TCH], f32r, tag="xm")
                            if io % 3 == 0:
                                nc.vector.tensor_tensor(
                                    xm_t[:], xc_t[:, io, :].bitcast(f32),
                                    m_t[:], A.mult)
                            else:
                                nc.gpsimd.tensor_tensor(
                                    xm_t[:], xc_t[:, io, :].bitcast(f32),
                                    m_t[:], A.mult)

                            st, sp = (io == 0), (io == IT - 1)
                            nc.tensor.matmul(z0[:], wgt[:, io, 0:128],
                                             xm_t[:], start=st, stop=sp)
                            nc.tensor.matmul(z1[:], wgt[:, io, 128:256],
                                             xm_t[:], start=st, stop=sp)
                            nc.tensor.matmul(cnt_ps[:], ones_t[:], m_t[:],
                                             start=(n == 0 and io == 0),
                                             stop=False)
                        for io in range(IT):
                            st, sp = (io == 0), (io == IT - 1)
                            nc.tensor.matmul(u0[:], wut[:, io, 0:128],
                                             xcb_t[:, io, :], start=st, stop=sp)
                            nc.tensor.matmul(u1[:], wut[:, io, 128:256],
                                             xcb_t[:, io, :], start=st, stop=sp)

                        for k, (zt, ut) in enumerate(((z0, u0), (z1, u1))):
                            gst = n * 2 + k
                            zs_t = zsp.tile([128, TCH], f32, tag="zs")
                            nc.scalar.activation(zs_t[:], zt[:], AF.Silu,
                                                 bias=pmu_t[:, gst:gst + 1])
                            us_t = usp.tile([128, TCH], f32, tag="us")
                            nc.scalar.add(us_t[:], ut[:], b2_t[:, gst:gst + 1])
                            az_t = azp.tile([128, TCH], f32, tag="tmp32")
                            nc.scalar.activation(az_t[:], zs_t[:], AF.Abs)
                            mm_t = mmp.tile([128, TCH], bf16, tag="mmk")
                            nc.vector.tensor_scalar(
                                mm_t[:], az_t[:], thm_t[:, gst:gst + 1],
                                None, A.is_gt)
                            t1_t = azp.tile([128, TCH], f32, tag="tmp32")
                            nc.gpsimd.tensor_tensor(t1_t[:], us_t[:], zs_t[:],
                                                    A.mult)
                            nc.vector.tensor_tensor(h_t[:, gst, :], t1_t[:],
                                                    mm_t[:], A.mult)
                            nc.tensor.matmul(
                                cnt_ps[:], sixt_t[:], mm_t[:], start=False,
                                stop=(n == NSL - 1 and k == 1))

                    act_sb = smp.tile([1, TCH], f32, tag="act")
                    nc.vector.tensor_scalar(act_sb[:], cnt_ps[:], 256.0, None,
                                            A.mult)
                    nc.scalar.dma_start(
                        bounce_in.ap()[ch, :, D_OUT:D_OUT + 1], act_sb[:])

                # ---- down matmul for this chunk: y[t, o] partial ----
                with ExitStack() as pctx:
                    yps = pctx.enter_context(
                        tc.tile_pool(name="yps", bufs=8, space="PSUM"))
                    for oh in range(OH):
                        yt = [[yps.tile([128, 512], f32,
                                        tag="y", name=f"y_{tt}_{oc}")
                               for oc in range(2)] for tt in range(TT)]
                        for st in range(STL):
                            slab = dwp.tile([128, OHW], bf16, tag="dws",
                                            name=f"dws_{ch}_{oh}_{st}")
                            nc.sync.dma_start(slab[:], dwt.ap()[oh, st])
                            for tt in range(TT):
                                lh = h_t[:, st, tt * 128:(tt + 1) * 128]
                                for oc in range(2):
                                    nc.tensor.matmul(
                                        yt[tt][oc][:], lh,
                                        slab[:, oc * 512:(oc + 1) * 512],
                                        start=(st == 0), stop=(st == STL - 1))
                        for tt in range(TT):
                            for oc in range(2):
                                ysb = ysp.tile([128, 512], f32, tag="ysb")
                                nc.scalar.copy(ysb[:], yt[tt][oc][:])
                                nc.scalar.dma_start(
                                    bounce_in.ap()[
                                        ch, tt * 128:(tt + 1) * 128,
                                        oh * OHW + oc * 512:
                                        oh * OHW + (oc + 1) * 512],
                                    ysb[:])

                if ch + 1 < NCH:
                    tiles = emit_loads(ch + 1)
                if ch == NCH - 1:
                    nc.gpsimd.collective_compute(
                        "ReduceScatter", A.add,
                        replica_groups=[list(range(W))],
                        ins=[bounce_in.ap()[ch]],
                        outs=[bounce_out.ap()[ch]])
                else:
                    pending_rs = ch

            # Drain RS results at the very end: these DMAs wait on collective
            # completion, and the Sync queue is FIFO -- placing them inside
            # the chunk loop stalls every later load behind the RS wait.
            for ch in range(NCH):
                nc.sync.dma_start(yout.ap()[ch], bounce_out.ap()[ch])

    nc.compile()
    return nc


def kernel(x, gate_weight, gate_thresholds, gate_mu, gate_std, up_weight,
           up_bias, mlp_thresholds, mlp_mad, down_weight):
    from concourse.bass_utils import run_bass_kernel_spmd

    x = np.asarray(x, np.float32)
    gate_weight = np.asarray(gate_weight, np.float32)
    gate_thresholds = np.asarray(gate_thresholds, np.float32)
    gate_mu = np.asarray(gate_mu, np.float32)
    gate_std = np.asarray(gate_std, np.float32)
    up_weight = np.asarray(up_weight, np.float32)
    up_bias = np.asarray(up_bias, np.float32)
    mlp_thresholds = np.asarray(mlp_thresholds, np.float32)
    mlp_mad = np.asarray(mlp_mad, np.float32)
    down_weight = np.asarray(down_weight, np.float32)

    if "nc" not in _cache:
        _cache["nc"] = _build()
    nc = _cache["nc"]

    xt = x.reshape(T, D_IN)
    xc = xt - gate_mu[None, :]                       # exact fp32, as reference
    xcT_blk = np.ascontiguousarray(xc.T.reshape(IT, 128, T))
    xcB_blk = xcT_blk.astype(ml_dtypes.bfloat16)

    thresh_g = gate_thresholds * gate_std[None, :]   # [NS, D_IN]
    post_mu = (gate_mu.astype(np.float64) @ gate_weight.astype(np.float64)
               ).astype(np.float32)
    bias2 = (up_bias.astype(np.float64)
             + gate_mu.astype(np.float64) @ up_weight.astype(np.float64)
             ).astype(np.float32)
    thm_full = mlp_thresholds * mlp_mad

    in_maps = []
    for c in range(W):
        sl = slice(c * SL, (c + 1) * SL)
        wgc = gate_weight[:, sl]
        wg_blk = np.ascontiguousarray(
            wgc.reshape(IT, 128, NSL, STRIPE).transpose(2, 1, 0, 3))
        wuc = up_weight[:, sl]
        wu_blk = np.ascontiguousarray(
            wuc.reshape(IT, 128, NSL, STRIPE).transpose(2, 1, 0, 3)
            .astype(ml_dtypes.bfloat16))
        dwc = down_weight[:, sl]                     # [D_OUT, SL]
        dwt_blk = np.ascontiguousarray(
            dwc.T.reshape(STL, 128, OH, OHW).transpose(2, 0, 1, 3)
            .astype(ml_dtypes.bfloat16))
        thg_blk = np.ascontiguousarray(
            thresh_g[c * NSL:(c + 1) * NSL]
            .reshape(NSL, IT, 128).transpose(2, 1, 0))
        nthg_blk = np.ascontiguousarray(-thg_blk)
        pmu_blk = np.ascontiguousarray(post_mu[sl].reshape(STL, 128).T)
        b2_blk = np.ascontiguousarray(bias2[sl].reshape(STL, 128).T)
        thm_blk = np.ascontiguousarray(thm_full[sl].reshape(STL, 128).T)
        in_maps.append(dict(
            xcT=xcT_blk, xcB=xcB_blk, wg=wg_blk, wu=wu_blk, dwt=dwt_blk,
            thg=thg_blk, nthg=nthg_blk, pmu=pmu_blk, b2=b2_blk, thm=thm_blk))

    trace = bool(os.environ.get("CWIC_TRACE"))
    res = run_bass_kernel_spmd(nc, in_maps, core_ids=list(range(W)),
                               trace=trace)
    if trace and res.exec_time_ns is not None:
        print(f"HW exec time: {res.exec_time_ns} ns")
        _cache["exec_time_ns"] = res.exec_time_ns
        _cache["trace"] = res.instructions_and_trace

    y = np.empty((T, D_OUT), np.float32)
    active = np.empty((T,), np.float32)
    for ch in range(NCH):
        for c in range(W):
            rows = slice(ch * TCH + c * TSH, ch * TCH + (c + 1) * TSH)
            shard = res.results[c]["yout"][ch]
            y[rows] = shard[:, :D_OUT]
            active[rows] = shard[:, D_OUT]

    dense_val = float(D_IN * D_INTER) + float((D_IN + D_OUT) * D_INTER)
    dense = np.full((B, S), dense_val, np.float32)
    return (y.reshape(B, S, D_OUT), dense, active.reshape(B, S))
